# revision 2
# baseline (speedup 1.0000x reference)
"""Trainium2 Bass kernel for EventPropLinear forward (LIF spiking layer).

Computes out[b,o,t]: spike trains of a leaky integrate-and-fire layer driven
by J = W @ x through double-exponential synapse/membrane dynamics:
    I[t] = a_s*I[t-1] + J[t];  V[t] = a_m*V[t-1] + b_m*I[t-1]
    spike = V > 1 -> V resets to 0.

Strategy (8 NeuronCores, data-parallel over batch, 16 samples/core):
  - GEMM on TensorE in bf16 2-split (W = W1 + W2, both bf16; x is 0/1 so
    exact in bf16) -> J accumulated in fp32 PSUM at full fp32-level accuracy.
  - I-recurrence via DVE tensor_tensor_scan along time, reading PSUM directly,
    b_m folded into the weights so the scan state is b_m*I.
  - V-recurrence (nonlinear reset) as a serial 2-op-per-step DVE loop over a
    [128 x 64] state tile (all 16*512 neurons of the core), reading/writing
    time-strided columns of one big SBUF buffer in place.
  - Spike extraction on ACT (Sign then Relu), in place, windowed per chunk.
  - DMA via SP HWDGE; bacc.Bacc legalizes semaphore waits.
"""
import numpy as np
import ml_dtypes
import concourse.bass as bass
import concourse.bacc as bacc
import concourse.mybir as mybir
import concourse.tile as tile
from concourse.bass_utils import run_bass_kernel_spmd
from contextlib import ExitStack

B, IN_DIM, OUT_DIM, STEPS = 128, 784, 512, 500
NCORES = 8
BL = B // NCORES            # 16 batches per core
KC, NK = 112, 7             # contraction chunking: 784 = 7*112
NC_ = OUT_DIM // 128        # 4 o-chunks
NSER = BL * NC_             # 64 series per core (series = (b, o-chunk))
TT = 512                    # padded time
CHUNKS = [160, 160, 128, 48, 16]
STRIDE = 520                # per-series column stride in the big buffer
a_m = 1.0 - 0.1 / 20.0      # 0.995
b_m = 0.1 / 20.0            # 0.005
a_s = 1.0 - 0.1 / 5.0       # 0.98
f32, bf16 = mybir.dt.float32, mybir.dt.bfloat16

_cache = {}


def _build(reps=1):
    nc = bacc.Bacc()
    wpk = nc.declare_dram_parameter("wpk", [KC, NK * 2 * NC_ * 128], bf16, isOutput=False)
    xp = nc.declare_dram_parameter("xp", [BL, IN_DIM, TT], bf16, isOutput=False)
    outp = nc.declare_dram_parameter("outp", [BL, OUT_DIM, STEPS], f32, isOutput=True)

    with tile.TileContext(nc) as tc, ExitStack() as ctx:
        sb = ctx.enter_context(tc.tile_pool(name="sb", bufs=1))
        xpool = ctx.enter_context(tc.tile_pool(name="xp", bufs=4))
        pspool = ctx.enter_context(tc.tile_pool(name="ps", bufs=8, space="PSUM"))

        wt = sb.tile([KC, NK * 2 * NC_ * 128], bf16, tag="wt")
        nc.sync.dma_start(wt[:], wpk[:, :])
        a_s_t = sb.tile([128, max(CHUNKS)], f32, tag="ast")
        nc.vector.memset(a_s_t[:], a_s)
        neg1 = sb.tile([128, 1], f32, tag="neg1")
        nc.vector.memset(neg1[:], -1.0)
        buf = sb.tile([128, NSER * STRIDE], f32, tag="buf")
        bufv = buf[:].rearrange("p (s t) -> p s t", s=NSER)
        MULT, ADD = mybir.AluOpType.mult, mybir.AluOpType.add
        ISLE, BYP = mybir.AluOpType.is_le, mybir.AluOpType.bypass
        vr = sb.tile([128, NSER], f32, tag="vr")
        carry = sb.tile([128, NSER], f32, tag="carry")

        for _rep in range(reps):
          nc.vector.memset(bufv[:, :, 0:2], 0.0)
          nc.vector.memset(vr[:], 0.0)
          t0 = 0
          for ci, CH in enumerate(CHUNKS):
              for b in range(BL):
                  xt = xpool.tile([KC, NK * CH], bf16, tag="xt")
                  nc.sync.dma_start(
                      xt[:].rearrange("p (k t) -> p k t", k=NK),
                      xp[b, :, t0:t0 + CH].rearrange("(k ki) t -> ki k t", ki=KC))
                  for c in range(NC_):
                      p = pspool.tile([128, CH], f32, tag="ps")
                      for k in range(NK):
                          for sp in range(2):
                              w0 = ((k * 2 + sp) * NC_ + c) * 128
                              nc.tensor.matmul(p[:], wt[:, w0:w0 + 128], xt[:, k * CH:(k + 1) * CH],
                                               start=(k == 0 and sp == 0),
                                               stop=(k == NK - 1 and sp == 1))
                      s = b * NC_ + c
                      base = s * STRIDE
                      init = 0.0 if ci == 0 else carry[:, s:s + 1]
                      nc.vector.tensor_tensor_scan(
                          buf[:, base + t0 + 2:base + t0 + 2 + CH], a_s_t[:, :CH], p[:],
                          init, MULT, ADD)
              # save chunk-boundary bI column before the V-loop overwrites it
              if ci + 1 < len(CHUNKS):
                  nc.vector.scalar_tensor_tensor(carry[:], bufv[:, :, t0 + CH + 1], 1.0,
                                                 bufv[:, :, t0 + CH + 1], MULT, BYP)
              # V steps for this chunk
              for t in range(t0 + 1, min(t0 + CH + 1, STEPS - 1)):
                  col = bufv[:, :, t + 1]
                  nc.vector.scalar_tensor_tensor(col, vr[:], a_m, col, MULT, ADD)
                  nc.vector.scalar_tensor_tensor(vr[:], col, 1.0, col, ISLE, MULT)
              # spikes (in place): window of V_new columns for this chunk
              w0c = 0 if ci == 0 else t0 + 2
              w1c = min(t0 + CH + 2, STEPS)
              if w1c > w0c:
                  win = bufv[:, :, w0c:w1c]
                  nc.scalar.activation(win, win, mybir.ActivationFunctionType.Sign,
                                       bias=neg1[:], scale=1.0)
                  nc.scalar.activation(win, win, mybir.ActivationFunctionType.Relu)
                  for b in range(BL):
                      src = bufv[:, b * NC_:(b + 1) * NC_, w0c:w1c]
                      dst = outp[b, :, w0c:w1c].rearrange("(c p) t -> p c t", p=128)
                      nc.sync.dma_start(dst, src)
              t0 += CH
    nc.finalize()
    return nc


def _prep_weights(weight):
    ws = (b_m * weight.astype(np.float64)).astype(np.float32)
    w1 = ws.astype(ml_dtypes.bfloat16)
    w2 = (ws - w1.astype(np.float32)).astype(ml_dtypes.bfloat16)
    wpk = np.zeros((KC, NK, 2, NC_, 128), ml_dtypes.bfloat16)
    for k in range(NK):
        for c in range(NC_):
            wpk[:, k, 0, c, :] = w1[c * 128:(c + 1) * 128, k * KC:(k + 1) * KC].T
            wpk[:, k, 1, c, :] = w2[c * 128:(c + 1) * 128, k * KC:(k + 1) * KC].T
    return np.ascontiguousarray(wpk.reshape(KC, -1))


def _in_maps(x, weight):
    wpk = _prep_weights(weight)
    xpad = np.zeros((B, IN_DIM, TT), ml_dtypes.bfloat16)
    xpad[:, :, :STEPS] = x.astype(ml_dtypes.bfloat16)
    return [{"wpk": wpk, "xp": np.ascontiguousarray(xpad[i * BL:(i + 1) * BL])}
            for i in range(NCORES)]


def kernel(x, weight):
    if "nc" not in _cache:
        _cache["nc"] = _build()
    nc = _cache["nc"]
    in_maps = _in_maps(x, weight)
    res = run_bass_kernel_spmd(nc, in_maps, list(range(NCORES)))
    out = np.concatenate([res.results[i]["outp"] for i in range(NCORES)], axis=0)
    return out.astype(np.float32)



# revision 10
# speedup vs baseline: 1.3999x; 1.3999x over previous
"""Trainium2 Bass kernel for EventPropLinear forward (LIF spiking layer).

Computes out[b,o,t]: spike trains of a leaky integrate-and-fire layer driven
by J = W @ x through double-exponential synapse/membrane dynamics:
    I[t] = a_s*I[t-1] + J[t];  V[t] = a_m*V[t-1] + b_m*I[t-1]
    spike = V > 1 -> V resets to 0.

Strategy (8 NeuronCores, data-parallel over batch, 16 samples/core):
  - GEMM on TensorE in bf16 2-split (W = W1 + W2, both bf16; x is 0/1 so
    exact in bf16) -> J in fp32 PSUM. Loops ordered (weight-tile, then 8
    batches) so each stationary load serves 8 matmuls.
  - J moved PSUM->SBUF by ScalarE (ACT) copies into a slotted ring
    ([sep | J(t0..t0+CH)] per series), then ONE DVE tensor_tensor_scan per
    chunk computes the I-recurrence for all 64 series in-place; a 0 in the
    decay pattern at each slot start reloads the carry exactly.
  - V-recurrence (nonlinear reset) via a custom DVE op (LIF_STEP_ANT):
    v' = t*(t<=1), t = a_m*v + bI  -- ONE instruction per time step for all
    64 series, in place over the ring. Reset-to-zero encodes spikes as v'==0.
  - Spike extraction on ACT: Sign(-v) then Relu(x+1) -> bf16 {0,1}; DMA out
    in bf16, host converts to fp32.
"""
import numpy as np
import ml_dtypes
import concourse.bass as bass
import concourse.bacc as bacc
import concourse.mybir as mybir
import concourse.tile as tile
from concourse.bass_utils import run_bass_kernel_spmd
from contextlib import ExitStack

B, IN_DIM, OUT_DIM, STEPS = 128, 784, 512, 500
NCORES = 8
BL = B // NCORES            # 16 samples per core
KC, NK = 112, 7             # contraction chunking: 784 = 7*112
NC_ = OUT_DIM // 128        # 4 o-chunks
NSER = BL * NC_             # 64 series per core (series = b*NC_ + c)
CH = 128                    # time chunk
NCHUNK = 4                  # chunks cover out index tau = 1..512
SL = CH + 1                 # ring slot: [sep | CH values]
TT = 1 + NCHUNK * CH        # xpad cols 0..512 (col tau holds x[tau-2])
a_m = 1.0 - 0.1 / 20.0      # 0.995
b_m = 0.1 / 20.0            # 0.005
a_s = 1.0 - 0.1 / 5.0       # 0.98
f32, bf16 = mybir.dt.float32, mybir.dt.bfloat16

_cache = {}


def _register_lif_step():
    from concourse.dve_spec import Spec, Src0, Src1, C0, One, lower
    from concourse.dve_uop import DveOpSpec
    from concourse.dve_ops import DveOp, OPS, CUSTOM_DVE_SPECS, _SUB_OPCODE_FOR_NAME

    name = "LIF_STEP_ANT"
    if name in _SUB_OPCODE_FOR_NAME:
        return next(o for o in OPS if o.name == name)
    t = Src1 * C0 + Src0
    spec = Spec(body=t * (t <= One),
                reference=lambda in0, in1, s0, s1, imm2:
                    (lambda tt: (tt * (tt <= 1.0)).astype(np.float32))(
                        in1 * np.float32(s0) + in0))
    row = 1 + len(OPS)
    _SUB_OPCODE_FOR_NAME[name] = row
    shas = {v: DveOpSpec(name=name, opcode=row, uops=lower(spec, ver=v),
                         rd1_en=True).sha(v) for v in ("v3", "v4")}
    op = DveOp(name, spec, subdim=False, uops_sha=shas)
    OPS.append(op)
    CUSTOM_DVE_SPECS[name] = spec
    return op


def _build():
    LIF_STEP = _register_lif_step()
    nc = bacc.Bacc()
    wpk = nc.declare_dram_parameter("wpk", [KC, NK * 2 * NC_ * 128], bf16, isOutput=False)
    xp = nc.declare_dram_parameter("xp", [BL, IN_DIM, TT], bf16, isOutput=False)
    outp = nc.declare_dram_parameter("outp", [BL, OUT_DIM, STEPS], bf16, isOutput=True)

    MULT, ADD = mybir.AluOpType.mult, mybir.AluOpType.add
    SIGN = mybir.ActivationFunctionType.Sign
    RELU = mybir.ActivationFunctionType.Relu
    COPY = mybir.ActivationFunctionType.Copy

    with tile.TileContext(nc) as tc, ExitStack() as ctx:
        sb = ctx.enter_context(tc.tile_pool(name="sb", bufs=1))
        xpool = ctx.enter_context(tc.tile_pool(name="xp", bufs=24))
        pspool = ctx.enter_context(tc.tile_pool(name="ps", bufs=8, space="PSUM"))

        wt = sb.tile([KC, NK * 2 * NC_ * 128], bf16, tag="wt")
        nc.sync.dma_start(wt[:], wpk[:, :])

        decay = sb.tile([128, NSER * SL], f32, tag="decay")
        nc.vector.memset(decay[:], a_s)
        dcv = decay[:].rearrange("p (s t) -> p s t", s=NSER)
        nc.vector.memset(dcv[:, :, 0], 0.0)

        ring0 = sb.tile([128, NSER * SL], f32, tag="ring0")
        ring1 = sb.tile([128, NSER * SL], f32, tag="ring1")
        spk0 = sb.tile([128, NSER * CH], bf16, tag="spk0")
        spk1 = sb.tile([128, NSER * CH], bf16, tag="spk1")
        rings = [ring0, ring1]
        rviews = [ring0[:].rearrange("p (s t) -> p s t", s=NSER),
                  ring1[:].rearrange("p (s t) -> p s t", s=NSER)]
        spks = [spk0, spk1]
        zerocol = sb.tile([128, NSER], f32, tag="zc")
        nc.vector.memset(zerocol[:], 0.0)
        vsave0 = sb.tile([128, NSER], f32, tag="vs0")
        vsave1 = sb.tile([128, NSER], f32, tag="vs1")
        vsaves = [vsave0, vsave1]
        # chunk 0 separators = 0 (I starts at 0)
        nc.vector.memset(rviews[0][:, :, 0], 0.0)

        for ci in range(NCHUNK):
            Li = 1 + ci * CH
            ring, rv = rings[ci % 2], rviews[ci % 2]
            spk = spks[ci % 2]
            valid = min(CH, STEPS - Li)      # V steps / output cols this chunk

            # ---- input DMA for this chunk (all 16 samples) ----
            xts = []
            for b in range(BL):
                xt = xpool.tile([KC, NK * CH], bf16, tag="xt")
                nc.sync.dma_start(
                    xt[:].rearrange("p (k t) -> p k t", k=NK),
                    xp[b, :, Li:Li + CH].rearrange("(k ki) t -> ki k t", ki=KC))
                xts.append(xt)

            # ---- GEMM + PSUM->ring copies (J for tau in [Li, Li+CH)) ----
            # One PSUM bank per sample: 4 c-series x CH cols. Weight-tile
            # loads amortize over the 8 samples of a half.
            for half in range(2):
                bs = list(range(half * 8, half * 8 + 8))
                pst = {}
                for b in bs:
                    pst[b] = pspool.tile([128, NC_ * CH], f32, tag="ps", name="ps")
                for c in range(NC_):
                    for ksp in range(NK * 2):
                        k, sp = ksp // 2, ksp % 2
                        w0 = ((k * 2 + sp) * NC_ + c) * 128
                        for b in bs:
                            nc.tensor.matmul(
                                pst[b][:, c * CH:(c + 1) * CH], wt[:, w0:w0 + 128],
                                xts[b][:, k * CH:(k + 1) * CH],
                                start=(ksp == 0), stop=(ksp == 2 * NK - 1))
                for b in bs:
                    nc.scalar.activation(
                        rv[:, b * NC_:(b + 1) * NC_, 1:1 + CH],
                        pst[b][:].rearrange("p (c t) -> p c t", c=NC_),
                        COPY, bias=0.0, scale=1.0)

            # ---- I-recurrence: one segmented in-place scan over all series ----
            nc.vector.tensor_tensor_scan(ring[:], decay[:], ring[:], 0.0, MULT, ADD)

            # ---- save next chunk's separators (carry) before V overwrites ----
            if ci + 1 < NCHUNK:
                nxt = rviews[(ci + 1) % 2]
                nc.scalar.activation(nxt[:, :, 0], rv[:, :, CH], COPY,
                                     bias=0.0, scale=1.0)

            # ---- V-recurrence: one fused DVE op per step ----
            for j in range(1, valid + 1):
                if ci == 0 and j == 1:
                    prev = zerocol[:]
                elif j == 1:
                    prev = vsaves[(ci - 1) % 2][:]
                else:
                    prev = rv[:, :, j - 1]
                nc.vector._custom_dve(LIF_STEP, out=rv[:, :, j], in0=rv[:, :, j],
                                      in1=prev, s0=a_m)

            # save boundary V column before extraction clobbers it in place
            if ci + 1 < NCHUNK:
                nc.scalar.activation(vsaves[ci % 2][:], rv[:, :, CH], COPY,
                                     bias=0.0, scale=1.0)

            # ---- spikes: v==0  ->  Relu(Sign(-v)+1) in bf16 ----
            win = rv[:, :, 1:1 + valid]
            nc.scalar.activation(win, win, SIGN, bias=0.0, scale=-1.0)
            spw = spk[:].rearrange("p (s t) -> p s t", s=NSER)[:, :, 0:valid]
            nc.scalar.activation(spw, win, RELU, bias=1.0, scale=1.0)

            # ---- output DMA (bf16) ----
            src = spk[:].rearrange("p (bb c t) -> p bb c t", bb=BL, c=NC_)[:, :, :, 0:valid]
            dst = outp[:, :, Li:Li + valid].rearrange("b (c p) t -> p b c t", p=128)
            nc.sync.dma_start(dst, src)
    nc.finalize()
    return nc


def _prep_weights(weight):
    ws = (b_m * weight.astype(np.float64)).astype(np.float32)
    w1 = ws.astype(ml_dtypes.bfloat16)
    w2 = (ws - w1.astype(np.float32)).astype(ml_dtypes.bfloat16)
    wpk = np.zeros((KC, NK, 2, NC_, 128), ml_dtypes.bfloat16)
    for k in range(NK):
        for c in range(NC_):
            wpk[:, k, 0, c, :] = w1[c * 128:(c + 1) * 128, k * KC:(k + 1) * KC].T
            wpk[:, k, 1, c, :] = w2[c * 128:(c + 1) * 128, k * KC:(k + 1) * KC].T
    return np.ascontiguousarray(wpk.reshape(KC, -1))


def _in_maps(x, weight):
    wpk = _prep_weights(weight)
    xpad = np.zeros((B, IN_DIM, TT), ml_dtypes.bfloat16)
    xpad[:, :, 2:2 + STEPS] = x.astype(ml_dtypes.bfloat16)
    return [{"wpk": wpk, "xp": np.ascontiguousarray(xpad[i * BL:(i + 1) * BL])}
            for i in range(NCORES)]


def kernel(x, weight):
    if "nc" not in _cache:
        _cache["nc"] = _build()
    nc = _cache["nc"]
    in_maps = _in_maps(x, weight)
    res = run_bass_kernel_spmd(nc, in_maps, list(range(NCORES)))
    out = np.concatenate([res.results[i]["outp"] for i in range(NCORES)],
                         axis=0).astype(np.float32)
    out[:, :, 0:2] = 0.0
    return out


# revision 13
# speedup vs baseline: 1.5075x; 1.0769x over previous
"""Trainium2 Bass kernel for EventPropLinear forward (LIF spiking layer).

Computes out[b,o,t]: spike trains of a leaky integrate-and-fire layer driven
by J = W @ x through double-exponential synapse/membrane dynamics:
    I[t] = a_s*I[t-1] + J[t];  V[t] = a_m*V[t-1] + b_m*I[t-1]
    spike = V > 1 -> V resets to 0.

Strategy (8 NeuronCores, data-parallel over batch, 16 samples/core):
  - GEMM on TensorE in bf16 2-split (W = W1 + W2, both bf16; x is 0/1 so
    exact in bf16) -> J in fp32 PSUM. Loops ordered (weight-tile, then 8
    batches) so each stationary load serves 8 matmuls.
  - J moved PSUM->SBUF by ScalarE (ACT) copies into a slotted ring
    ([sep | J(t0..t0+CH)] per series), then ONE DVE tensor_tensor_scan per
    chunk computes the I-recurrence for all 64 series in-place; a 0 in the
    decay pattern at each slot start reloads the carry exactly.
  - V-recurrence (nonlinear reset) via a custom DVE op (LIF_STEP_ANT):
    v' = t*(t<=1), t = a_m*v + bI  -- ONE instruction per time step for all
    64 series, in place over the ring. Reset-to-zero encodes spikes as v'==0.
  - Spike extraction on ACT: Sign(-v) then Relu(x+1) -> bf16 {0,1}; DMA out
    in bf16, host converts to fp32.
"""
import numpy as np
import ml_dtypes
import concourse.bass as bass
import concourse.bacc as bacc
import concourse.mybir as mybir
import concourse.tile as tile
from concourse.bass_utils import run_bass_kernel_spmd
from contextlib import ExitStack

B, IN_DIM, OUT_DIM, STEPS = 128, 784, 512, 500
NCORES = 8
BL = B // NCORES            # 16 samples per core
KC, NK = 112, 7             # contraction chunking: 784 = 7*112
NC_ = OUT_DIM // 128        # 4 o-chunks
NSER = BL * NC_             # 64 series per core (series = b*NC_ + c)
CH = 128                    # time chunk
NCHUNK = 4                  # chunks cover out index tau = 1..512
SL = CH + 1                 # ring slot: [sep | CH values]
TT = 1 + NCHUNK * CH        # xpad cols 0..512 (col tau holds x[tau-2])
a_m = 1.0 - 0.1 / 20.0      # 0.995
b_m = 0.1 / 20.0            # 0.005
a_s = 1.0 - 0.1 / 5.0       # 0.98
f32, bf16 = mybir.dt.float32, mybir.dt.bfloat16

_cache = {}


def _register_lif_step():
    from concourse.dve_spec import Spec, Src0, Src1, C0, One, lower
    from concourse.dve_uop import DveOpSpec
    from concourse.dve_ops import DveOp, OPS, CUSTOM_DVE_SPECS, _SUB_OPCODE_FOR_NAME

    name = "LIF_STEP_ANT"
    if name in _SUB_OPCODE_FOR_NAME:
        return next(o for o in OPS if o.name == name)
    t = Src1 * C0 + Src0
    spec = Spec(body=t * (t <= One),
                reference=lambda in0, in1, s0, s1, imm2:
                    (lambda tt: (tt * (tt <= 1.0)).astype(np.float32))(
                        in1 * np.float32(s0) + in0))
    row = 1 + len(OPS)
    _SUB_OPCODE_FOR_NAME[name] = row
    shas = {v: DveOpSpec(name=name, opcode=row, uops=lower(spec, ver=v),
                         rd1_en=True).sha(v) for v in ("v3", "v4")}
    op = DveOp(name, spec, subdim=False, uops_sha=shas)
    OPS.append(op)
    CUSTOM_DVE_SPECS[name] = spec
    return op


def _build():
    LIF_STEP = _register_lif_step()
    nc = bacc.Bacc()
    wpk = nc.declare_dram_parameter("wpk", [KC, NK * 2 * NC_ * 128], bf16, isOutput=False)
    xp = nc.declare_dram_parameter("xp", [BL, IN_DIM, TT], bf16, isOutput=False)
    outp = nc.declare_dram_parameter("outp", [BL, OUT_DIM, STEPS], bf16, isOutput=True)

    MULT, ADD = mybir.AluOpType.mult, mybir.AluOpType.add
    SIGN = mybir.ActivationFunctionType.Sign
    RELU = mybir.ActivationFunctionType.Relu
    COPY = mybir.ActivationFunctionType.Copy

    with tile.TileContext(nc) as tc, ExitStack() as ctx:
        sb = ctx.enter_context(tc.tile_pool(name="sb", bufs=1))
        xpool = ctx.enter_context(tc.tile_pool(name="xp", bufs=24))
        pspool = ctx.enter_context(tc.tile_pool(name="ps", bufs=8, space="PSUM"))

        wt = sb.tile([KC, NK * 2 * NC_ * 128], bf16, tag="wt")
        nc.sync.dma_start(wt[:], wpk[:, :])

        decay = sb.tile([128, NSER * SL], f32, tag="decay")
        nc.vector.memset(decay[:], a_s)
        dcv = decay[:].rearrange("p (s t) -> p s t", s=NSER)
        nc.vector.memset(dcv[:, :, 0], 0.0)

        ring0 = sb.tile([128, NSER * SL], f32, tag="ring0")
        ring1 = sb.tile([128, NSER * SL], f32, tag="ring1")
        spk0 = sb.tile([128, NSER * CH], bf16, tag="spk0")
        spk1 = sb.tile([128, NSER * CH], bf16, tag="spk1")
        rings = [ring0, ring1]
        rviews = [ring0[:].rearrange("p (s t) -> p s t", s=NSER),
                  ring1[:].rearrange("p (s t) -> p s t", s=NSER)]
        spks = [spk0, spk1]
        zerocol = sb.tile([128, NSER], f32, tag="zc")
        nc.vector.memset(zerocol[:], 0.0)
        vsave0 = sb.tile([128, NSER], f32, tag="vs0")
        vsave1 = sb.tile([128, NSER], f32, tag="vs1")
        vsaves = [vsave0, vsave1]
        # chunk 0 separators = 0 (I starts at 0)
        nc.vector.memset(rviews[0][:, :, 0], 0.0)

        def extract_and_dma(cj):
            # spikes: v==0 -> Relu(Sign(-v)+1) in bf16, then DMA out
            Lj = 1 + cj * CH
            vj = min(CH, STEPS - Lj)
            rvj, spkj = rviews[cj % 2], spks[cj % 2]
            win = rvj[:, :, 1:1 + vj]
            nc.scalar.activation(win, win, SIGN, bias=0.0, scale=-1.0)
            spw = spkj[:].rearrange("p (s t) -> p s t", s=NSER)[:, :, 0:vj]
            nc.scalar.activation(spw, win, RELU, bias=1.0, scale=1.0)
            src = spkj[:].rearrange("p (bb c t) -> p bb c t",
                                    bb=BL, c=NC_)[:, :, :, 0:vj]
            dst = outp[:, :, Lj:Lj + vj].rearrange("b (c p) t -> p b c t", p=128)
            nc.sync.dma_start(dst, src)

        for ci in range(NCHUNK):
            Li = 1 + ci * CH
            ring, rv = rings[ci % 2], rviews[ci % 2]
            valid = min(CH, STEPS - Li)      # V steps / output cols this chunk

            # ---- input DMA for this chunk (all 16 samples) ----
            xts = []
            for b in range(BL):
                xt = xpool.tile([KC, NK * CH], bf16, tag="xt")
                nc.sync.dma_start(
                    xt[:].rearrange("p (k t) -> p k t", k=NK),
                    xp[b, :, Li:Li + CH].rearrange("(k ki) t -> ki k t", ki=KC))
                xts.append(xt)

            # ---- GEMM + PSUM->ring copies (J for tau in [Li, Li+CH)) ----
            # One PSUM bank per sample: 4 c-series x CH cols. Weight-tile
            # loads amortize over the 8 samples of a half.
            for half in range(2):
                bs = list(range(half * 8, half * 8 + 8))
                pst = {}
                for b in bs:
                    pst[b] = pspool.tile([128, NC_ * CH], f32, tag="ps", name="ps")
                for c in range(NC_):
                    for ksp in range(NK * 2):
                        k, sp = ksp // 2, ksp % 2
                        w0 = ((k * 2 + sp) * NC_ + c) * 128
                        for b in bs:
                            nc.tensor.matmul(
                                pst[b][:, c * CH:(c + 1) * CH], wt[:, w0:w0 + 128],
                                xts[b][:, k * CH:(k + 1) * CH],
                                start=(ksp == 0), stop=(ksp == 2 * NK - 1))
                for b in bs:
                    nc.scalar.activation(
                        rv[:, b * NC_:(b + 1) * NC_, 1:1 + CH],
                        pst[b][:].rearrange("p (c t) -> p c t", c=NC_),
                        COPY, bias=0.0, scale=1.0)
                # I-recurrence for this half's 32 series (segmented in-place scan)
                h0 = half * 32 * SL
                nc.vector.tensor_tensor_scan(
                    ring[:, h0:h0 + 32 * SL], decay[:, h0:h0 + 32 * SL],
                    ring[:, h0:h0 + 32 * SL], 0.0, MULT, ADD)

            # ---- save next chunk's separators (carry) before V overwrites ----
            if ci + 1 < NCHUNK:
                nxt = rviews[(ci + 1) % 2]
                nc.scalar.activation(nxt[:, :, 0], rv[:, :, CH], COPY,
                                     bias=0.0, scale=1.0)

            # ---- V-recurrence: one fused DVE op per step ----
            for j in range(1, valid + 1):
                if ci == 0 and j == 1:
                    prev = zerocol[:]
                elif j == 1:
                    prev = vsaves[(ci - 1) % 2][:]
                else:
                    prev = rv[:, :, j - 1]
                nc.vector._custom_dve(LIF_STEP, out=rv[:, :, j], in0=rv[:, :, j],
                                      in1=prev, s0=a_m)

            # save boundary V column before extraction clobbers it in place
            if ci + 1 < NCHUNK:
                nc.scalar.activation(vsaves[ci % 2][:], rv[:, :, CH], COPY,
                                     bias=0.0, scale=1.0)

            # extraction of the PREVIOUS chunk (lag-1: keeps this chunk's
            # J-copies from queuing behind it on the scalar engine)
            if ci >= 1:
                extract_and_dma(ci - 1)
        extract_and_dma(NCHUNK - 1)
    nc.finalize()
    return nc


def _prep_weights(weight):
    ws = (b_m * weight.astype(np.float64)).astype(np.float32)
    w1 = ws.astype(ml_dtypes.bfloat16)
    w2 = (ws - w1.astype(np.float32)).astype(ml_dtypes.bfloat16)
    wpk = np.zeros((KC, NK, 2, NC_, 128), ml_dtypes.bfloat16)
    for k in range(NK):
        for c in range(NC_):
            wpk[:, k, 0, c, :] = w1[c * 128:(c + 1) * 128, k * KC:(k + 1) * KC].T
            wpk[:, k, 1, c, :] = w2[c * 128:(c + 1) * 128, k * KC:(k + 1) * KC].T
    return np.ascontiguousarray(wpk.reshape(KC, -1))


def _in_maps(x, weight):
    wpk = _prep_weights(weight)
    xpad = np.zeros((B, IN_DIM, TT), ml_dtypes.bfloat16)
    xpad[:, :, 2:2 + STEPS] = x.astype(ml_dtypes.bfloat16)
    return [{"wpk": wpk, "xp": np.ascontiguousarray(xpad[i * BL:(i + 1) * BL])}
            for i in range(NCORES)]


def kernel(x, weight):
    if "nc" not in _cache:
        _cache["nc"] = _build()
    nc = _cache["nc"]
    in_maps = _in_maps(x, weight)
    res = run_bass_kernel_spmd(nc, in_maps, list(range(NCORES)))
    out = np.concatenate([res.results[i]["outp"] for i in range(NCORES)],
                         axis=0).astype(np.float32)
    out[:, :, 0:2] = 0.0
    return out
